# revision 27
# baseline (speedup 1.0000x reference)
"""Strided (residue-group) attention for Trainium2, SPMD across 8 NeuronCores.

Problem: x[B=2,S=4096,E=1024] -> qkv proj -> per-(batch,head,residue-group)
attention (stride 8 -> 8 groups of n=512 tokens) -> out proj.

Sharding: by (batch, residue-group).  B*stride = 16 group-instances; each of
the 8 cores owns 2 (batch,group) pairs = 1024 tokens and computes their FULL
output rows (it holds all 16 heads for its tokens).  The residue groups are
independent, so there are no cross-device collectives at all; the host
permutes tokens into group-major order on the way in and inverts on the way
out.

Device kernel design (per core):
  - Host pre-transposes x so the kernel receives xT [E, 1024tok].
  - QKV: qT,kT produced feature-on-partition ([f,tok]); v produced
    token-on-partition ([tok,f]).  fp16 inputs, f32 PSUM.
  - scoresT[k,q] = kT.T-chunks @ qT per head; head pairs row-packed on the
    PE array (K=64 each at rows 0-63 / 64-127, separate PSUM banks).
  - exp on ScalarE without max-subtraction (scores are O(+-8), exp is safe).
    ACT runs ONLY Exp -> exactly one activation-table load.
  - PV: lhsT = [v | ones] (even heads) or [ones | v] (odd heads) so one
    accumulation chain yields o-rows plus 64 replicated softmax-denominator
    rows for free (matmul cost depends only on the moving free size).
  - softmax normalize: the denominator half of the PV PSUM is copied to the
    o-row partitions of an SBUF tile by a small partition-swap DMA (f32),
    one DVE reciprocal_approx_fast per pair, then two DVE multiplies.  No
    ACT ln/exp, no table thrash, no gpsimd broadcast.
  - out proj: lhsT = oT chunks, rhs = Wout rows -> natural [tok, E] output.
  - Software pipeline: scores(pair) and PV(pair) are separated in PE program
    order by an interleaved QKV/outproj unit so the in-order PE never waits
    on the ACT exp chain; DMAs are spread over the sync/gpsimd/vector queues.
"""

import os

import numpy as np

B, S, E = 2, 4096, 1024
H, ST = 16, 8
DH = E // H  # 64
N = S // ST  # 512 tokens per residue group
NCORES = 8
GPC = (B * ST) // NCORES  # 2 (batch,group) pairs per core
TOK = GPC * N  # 1024 tokens per core
P = 128
EC = E // P  # 8 contraction chunks of 128
NB = N // P  # 4 token chunks of 128 per group
FB = 2  # feature blocks of 512 in E
SCALE = 1.0 / float(np.sqrt(DH))

_CACHE: dict = {}


def _build_nc():
    import concourse.bass as bass
    import concourse.bacc as bacc
    import concourse.tile as tile
    from concourse import mybir

    F32 = mybir.dt.float32
    FP16 = mybir.dt.float16
    ADD = mybir.AluOpType.add
    EXP = mybir.ActivationFunctionType.Exp

    nc = bacc.Bacc()
    xt = nc.declare_dram_parameter("xt", [E, TOK], FP16, isOutput=False)
    wq = nc.declare_dram_parameter("wq", [EC, P, EC, P], FP16, isOutput=False)
    wk = nc.declare_dram_parameter("wk", [EC, P, EC, P], FP16, isOutput=False)
    wv = nc.declare_dram_parameter("wv", [E, E], FP16, isOutput=False)
    wo = nc.declare_dram_parameter("wo", [E, E], FP16, isOutput=False)
    bq = nc.declare_dram_parameter("bq", [E], F32, isOutput=False)
    bk = nc.declare_dram_parameter("bk", [E], F32, isOutput=False)
    bv = nc.declare_dram_parameter("bv", [E], F32, isOutput=False)
    bo = nc.declare_dram_parameter("bo", [E], F32, isOutput=False)
    out = nc.declare_dram_parameter("out", [TOK, E], F32, isOutput=True)

    with tile.TileContext(nc) as tc, (
        tc.tile_pool(name="const", bufs=1)
    ) as const, tc.tile_pool(name="xtp", bufs=5) as xtp, tc.tile_pool(
        name="wqkp", bufs=4
    ) as wqkp, tc.tile_pool(name="wvp", bufs=3) as wvp, tc.tile_pool(
        name="qtp", bufs=9
    ) as qtp, tc.tile_pool(name="ktp", bufs=9) as ktp, tc.tile_pool(
        name="vpp", bufs=5
    ) as vpp, tc.tile_pool(name="expp", bufs=4) as expp, tc.tile_pool(
        name="osbp", bufs=6
    ) as osbp, tc.tile_pool(name="recp", bufs=3) as recp, tc.tile_pool(
        name="rcfp", bufs=3
    ) as rcfp, tc.tile_pool(name="otp", bufs=17) as otp, tc.tile_pool(
        name="outp", bufs=3
    ) as outp, tc.tile_pool(name="psmm", bufs=2, space="PSUM") as psmm, tc.tile_pool(
        name="pssc", bufs=2, space="PSUM"
    ) as pssc, tc.tile_pool(name="pso", bufs=2, space="PSUM") as psop:
        # ---- constants (vector queue; off the sync/gpsimd critical path) --
        bq_sb = const.tile([P, EC], F32)
        nc.scalar.dma_start(out=bq_sb, in_=bq[:].rearrange("(c p) -> p c", p=P))
        bk_sb = const.tile([P, EC], F32)
        nc.scalar.dma_start(out=bk_sb, in_=bk[:].rearrange("(c p) -> p c", p=P))
        bv_bc = const.tile([P, E], F32)
        nc.scalar.dma_start(out=bv_bc, in_=bv[:].partition_broadcast(P))
        bo_bc = const.tile([P, E], F32)
        nc.scalar.dma_start(out=bo_bc, in_=bo[:].partition_broadcast(P))
        # Wout resident (fp16): [p, fb, dc, 512]; loaded mid-pipeline
        wo_sb = const.tile([P, FB, EC, 512], FP16)

        def load_wo():
            for fb in range(FB):
                nc.gpsimd.dma_start(
                    out=wo_sb[:, fb],
                    in_=wo[:, fb * 512 : (fb + 1) * 512].rearrange(
                        "(c p) f -> p c f", p=P
                    ),
                )

        xt_c = {0: [], 1: []}
        qts = {0: [], 1: []}
        kts = {0: [], 1: []}
        vts = {0: [], 1: []}
        ots = {0: [], 1: []}
        exs = {}
        recs = {}
        osbs = {0: {}, 1: {}}

        # xt: two batched DMAs per group ([P, 4, N], 4KB/partition) so the
        # QKV chain is not throttled by per-chunk DMA issue serialization
        def load_xt_g(g):
            eng = nc.sync if g == 0 else nc.gpsimd
            pieces = 2
            cpp = EC // pieces
            for piece in range(pieces):
                t = xtp.tile([P, cpp, N], FP16, tag="xt")
                eng.dma_start(
                    out=t,
                    in_=xt[
                        piece * cpp * P : (piece + 1) * cpp * P,
                        g * N : (g + 1) * N,
                    ].rearrange("(c p) n -> p c n", p=P),
                )
                for c in range(cpp):
                    xt_c[g].append(t[:, c, :])

        def emit_qk_ftile(g, which, ft):
            wmat, bias_sb, lst = (
                (wq, bq_sb, qts[g]) if which == "q" else (wk, bk_sb, kts[g])
            )
            wt = wqkp.tile([P, EC, P], FP16, tag="wqk")
            nc.gpsimd.dma_start(out=wt, in_=wmat[ft])
            ps = psmm.tile([P, N], F32, tag="mm")
            for c in range(EC):
                nc.tensor.matmul(
                    ps,
                    lhsT=wt[:, c, :],
                    rhs=xt_c[g][c],
                    start=(c == 0),
                    stop=(c == EC - 1),
                )
            if which == "q":
                t = qtp.tile([P, N], FP16, tag="qt")
            else:
                t = ktp.tile([P, N], FP16, tag="kt")
            # evac+bias on ScalarE (Identity shares the Exp act table, so no
            # table reload); keeps the in-order DVE stream off the PE's
            # scores-weight critical path
            nc.scalar.activation(
                out=t,
                in_=ps,
                func=mybir.ActivationFunctionType.Identity,
                bias=bias_sb[:, ft : ft + 1],
            )
            lst.append(t)

        def emit_qk_unit(g, ft):
            emit_qk_ftile(g, "q", ft)
            emit_qk_ftile(g, "k", ft)

        def emit_v_fb(g, fb):
            if fb == 0:
                for tt in range(NB):
                    vt = vpp.tile([P, H, P], FP16, tag="vp")
                    # ones blocks (even heads: cols 64-127; odd: cols 0-63);
                    # the v-projection writes the v halves
                    v2 = vt.rearrange("p (h2 two) d -> p h2 two d", two=2)
                    nc.gpsimd.memset(v2[:, :, 0, DH:P], 1.0)
                    nc.gpsimd.memset(v2[:, :, 1, 0:DH], 1.0)
                    vts[g].append(vt)
            wvt = wvp.tile([P, EC, 512], FP16, tag="wv")
            nc.gpsimd.dma_start(
                out=wvt,
                in_=wv[:, fb * 512 : (fb + 1) * 512].rearrange(
                    "(c p) f -> p c f", p=P
                ),
            )
            wv_ts = [wvt[:, c, :] for c in range(EC)]
            for tt in range(NB):
                ps = psmm.tile([P, 512], F32, tag="mm")
                for c in range(EC):
                    nc.tensor.matmul(
                        ps,
                        lhsT=xt_c[g][c][:, tt * P : (tt + 1) * P],
                        rhs=wv_ts[c],
                        start=(c == 0),
                        stop=(c == EC - 1),
                    )
                # batched bias-add + scatter into the per-head v layout:
                # heads fb*8+hl; even hl -> v at cols 0:64, odd -> 64:128.
                vt = vts[g][tt]
                v2 = vt.rearrange("p (h2 two) d -> p h2 two d", two=2)
                p2 = ps.rearrange("p (f2 two d) -> p f2 two d", two=2, d=DH)
                b2 = bv_bc[:, fb * 512 : (fb + 1) * 512].rearrange(
                    "p (f2 two d) -> p f2 two d", two=2, d=DH
                )
                h2lo = fb * (H // 4)
                h2hi = (fb + 1) * (H // 4)
                nc.vector.tensor_add(
                    out=v2[:, h2lo:h2hi, 0, 0:DH],
                    in0=p2[:, :, 0, :],
                    in1=b2[:, :, 0, :],
                )
                nc.vector.tensor_add(
                    out=v2[:, h2lo:h2hi, 1, DH:P],
                    in0=p2[:, :, 1, :],
                    in1=b2[:, :, 1, :],
                )

        def emit_scores_pair(g, pr):
            # scores + exp for both heads of the pair, row-packed on the PE
            # array (K=64 each at rows 0-63 / 64-127, separate PSUM banks)
            ex_AB = {}
            for h in (2 * pr, 2 * pr + 1):
                ex_AB[h] = expp.tile([P, NB, N], FP16, tag="exp", name=f"ex{g}_{h}")
            for half in range(2):
                scs = {}
                for h in (2 * pr, 2 * pr + 1):
                    lo, hi = (0, DH) if h % 2 == 0 else (DH, P)
                    sc = pssc.tile([P, 2, N], F32, tag="sc")
                    for cc in range(2):
                        c = 2 * half + cc
                        nc.tensor.matmul(
                            sc[:, cc],
                            lhsT=kts[g][pr][lo:hi, c * P : (c + 1) * P],
                            rhs=qts[g][pr][lo:hi, :],
                            start=True,
                            stop=True,
                        )
                    scs[h] = sc
                for h in (2 * pr, 2 * pr + 1):
                    nc.scalar.activation(
                        out=ex_AB[h][:, 2 * half : 2 * half + 2],
                        in_=scs[h],
                        func=EXP,
                    )
            exs[(g, pr)] = ex_AB

        def emit_pv_pair(g, pr):
            ex_AB = exs.pop((g, pr))
            for h in (2 * pr, 2 * pr + 1):
                ex = ex_AB[h]
                po = psop.tile([P, N], F32, tag="po")
                for c in range(NB):
                    nc.tensor.matmul(
                        po,
                        lhsT=vts[g][c][:, h, :],
                        rhs=ex[:, c, :],
                        start=(c == 0),
                        stop=(c == NB - 1),
                    )
                osb = osbp.tile([P, N], F32, tag="osb")
                nc.vector.tensor_copy(out=osb, in_=po)
                osbs[g][h] = osb

        def emit_swap_pair(g, pr):
            # partition-swap DMAs: move the replicated denominator rows onto
            # the o-row partitions of the rec tile (SBUF -> SBUF).  Emitted a
            # full pair-slot after the evacuations so the in-order sync queue
            # head never blocks waiting for them.
            rec = recp.tile([P, N], F32, tag="rec")
            nc.sync.dma_start(out=rec[0:DH, :], in_=osbs[g][2 * pr][DH:P, :])
            nc.sync.dma_start(out=rec[DH:P, :], in_=osbs[g][2 * pr + 1][0:DH, :])
            recs[(g, pr)] = rec

        def emit_norm_pair(g, pr):
            # reciprocal on DVE; the normalize multiplies run on the mostly
            # idle GpSimd engine so they never serialize the DVE stream
            rec = recs.pop((g, pr))
            rcf = rcfp.tile([P, N], F32, tag="rcf")
            nc.vector.reciprocal_approx_fast(out=rcf, in_=rec)
            ot = otp.tile([P, N], FP16, tag="ot")
            he, ho = 2 * pr, 2 * pr + 1
            nc.gpsimd.tensor_mul(
                out=ot[0:DH, :], in0=osbs[g][he][0:DH, :], in1=rcf[0:DH, :]
            )
            nc.gpsimd.tensor_mul(
                out=ot[DH:P, :], in0=osbs[g][ho][DH:P, :], in1=rcf[DH:P, :]
            )
            del osbs[g][he], osbs[g][ho]
            ots[g].append(ot)

        def emit_slot(g, pr, mids=()):
            # one software-pipeline slot: scores(pr), then the previous
            # pair's swap DMAs (their evac deps resolved last slot, so the
            # sync queue head never blocks), the interleaved PE units, the
            # previous pair's normalize (its swap completed mid-slot), and
            # finally PV(pr), which waits on this pair's exp chain that ran
            # under the interleaved units.
            emit_scores_pair(g, pr)
            if pr > 0:
                emit_swap_pair(g, pr - 1)
            for m in mids:
                m()
            if pr > 0:
                emit_norm_pair(g, pr - 1)
            emit_pv_pair(g, pr)

        def emit_outproj_unit(g, fb, tt):
            ps = psmm.tile([P, 512], F32, tag="mm")
            for dc in range(EC):
                nc.tensor.matmul(
                    ps,
                    lhsT=ots[g][dc][:, tt * P : (tt + 1) * P],
                    rhs=wo_sb[:, fb, dc, :],
                    start=(dc == 0),
                    stop=(dc == EC - 1),
                )
            ob = outp.tile([P, 512], F32, tag="ob")
            nc.vector.tensor_add(
                out=ob, in0=ps, in1=bo_bc[:, fb * 512 : (fb + 1) * 512]
            )
            nc.gpsimd.dma_start(
                out=out[
                    g * N + tt * P : g * N + (tt + 1) * P,
                    fb * 512 : (fb + 1) * 512,
                ],
                in_=ob,
            )

        # ---- software-pipelined program order --------------------------
        load_xt_g(0)
        emit_qk_unit(0, 0)
        emit_qk_unit(0, 1)
        emit_slot(0, 0, [lambda: emit_v_fb(0, 0)])
        emit_slot(0, 1, [lambda: emit_qk_unit(0, 2)])
        emit_slot(0, 2, [lambda: emit_qk_unit(0, 3)])
        emit_slot(0, 3, [lambda: emit_qk_unit(0, 4), lambda: emit_qk_unit(0, 5)])
        emit_slot(0, 4, [lambda: load_xt_g(1), lambda: emit_v_fb(0, 1)])
        emit_slot(0, 5, [lambda: emit_qk_unit(0, 6), load_wo])
        emit_slot(0, 6, [lambda: emit_qk_unit(0, 7)])
        emit_slot(0, 7, [lambda: emit_qk_unit(1, 0)])
        # group-0 tail: swap+norm un-deferred (nothing else to overlap)
        emit_qk_unit(1, 1)
        emit_swap_pair(0, 7)
        emit_qk_unit(1, 2)
        emit_norm_pair(0, 7)
        # group-1 qk projection (PE-dense, ACT idle)
        for ft in range(3, EC):
            emit_qk_unit(1, ft)
        # group-1 attention interleaved with group-0 out proj
        emit_slot(1, 0, [lambda: emit_v_fb(1, 0)])
        emit_slot(1, 1, [lambda: emit_outproj_unit(0, 0, 0)])
        emit_slot(1, 2, [lambda: emit_outproj_unit(0, 0, 1)])
        emit_slot(
            1,
            3,
            [lambda: emit_outproj_unit(0, 0, 2), lambda: emit_outproj_unit(0, 0, 3)],
        )
        emit_slot(1, 4, [lambda: emit_v_fb(1, 1)])
        emit_slot(
            1,
            5,
            [lambda: emit_outproj_unit(0, 1, 0), lambda: emit_outproj_unit(0, 1, 1)],
        )
        emit_slot(1, 6, [lambda: emit_outproj_unit(0, 1, 2)])
        emit_slot(1, 7, [lambda: emit_outproj_unit(0, 1, 3)])
        # group-1 tail: swap+norm un-deferred
        emit_swap_pair(1, 7)
        emit_norm_pair(1, 7)
        for fb in range(FB):
            for tt in range(NB):
                emit_outproj_unit(1, fb, tt)
    nc.finalize()
    return nc


def _get_nc():
    if "nc" not in _CACHE:
        _CACHE["nc"] = _build_nc()
    return _CACHE["nc"]


def _make_in_maps(x, Wqkv, bqkv, Wout, bout):
    """Host-side sharding: permute tokens to group-major, pre-transpose x."""
    x = np.asarray(x, dtype=np.float32)
    Wqkv = np.asarray(Wqkv, dtype=np.float32)
    bqkv = np.asarray(bqkv, dtype=np.float32)
    Wout = np.ascontiguousarray(np.asarray(Wout, dtype=np.float16))
    bout = np.ascontiguousarray(np.asarray(bout, dtype=np.float32))

    # group-major token order: x_perm[b, g*N + i] = x[b, i*ST + g]
    x_perm = x.reshape(B, N, ST, E).transpose(0, 2, 1, 3)  # [B, ST, N, E]

    # [E, E] -> [ft, p, c, f] tile-major so each SBUF partition reads big runs
    def tile_qk(w):
        return np.ascontiguousarray(
            w.reshape(EC, P, EC, P).transpose(2, 1, 0, 3).astype(np.float16)
        )

    wq = tile_qk(Wqkv[:, 0:E] * SCALE)
    wk = tile_qk(Wqkv[:, E : 2 * E])
    wv = np.ascontiguousarray(Wqkv[:, 2 * E : 3 * E].astype(np.float16))
    bq = np.ascontiguousarray(bqkv[0:E] * SCALE)
    bk = np.ascontiguousarray(bqkv[E : 2 * E])
    bv = np.ascontiguousarray(bqkv[2 * E : 3 * E])

    in_maps = []
    for c in range(NCORES):
        b = c // (NCORES // B)
        g0 = GPC * (c % (NCORES // B))
        xc = x_perm[b, g0 : g0 + GPC].reshape(TOK, E)  # [1024, E]
        xct = np.ascontiguousarray(xc.T.astype(np.float16))  # [E, 1024]
        in_maps.append(
            {
                "xt": xct,
                "wq": wq,
                "wk": wk,
                "wv": wv,
                "wo": Wout,
                "bq": bq,
                "bk": bk,
                "bv": bv,
                "bo": bout,
            }
        )
    return in_maps


def kernel(x, Wqkv, bqkv, Wout, bout):
    from concourse.bass_utils import run_bass_kernel_spmd

    nc = _get_nc()
    in_maps = _make_in_maps(x, Wqkv, bqkv, Wout, bout)
    trace = bool(int(os.environ.get("KERNEL_TRACE", "0")))
    res = run_bass_kernel_spmd(
        nc, in_maps, core_ids=list(range(NCORES)), trace=trace
    )
    _CACHE["last_result"] = res

    # reassemble: core outputs are [1024 tok, E] in group-major token order
    out = np.empty((B, S, E), dtype=np.float32)
    for b in range(B):
        per_b = [res.results[b * (NCORES // B) + j]["out"] for j in range(NCORES // B)]
        perm = np.concatenate(per_b, axis=0)  # [ST*N, E] group-major
        out[b] = perm.reshape(ST, N, E).transpose(1, 0, 2).reshape(S, E)
    return out


# revision 29
# speedup vs baseline: 1.0114x; 1.0114x over previous
"""Strided (residue-group) attention for Trainium2, SPMD across 8 NeuronCores.

Problem: x[B=2,S=4096,E=1024] -> qkv proj -> per-(batch,head,residue-group)
attention (stride 8 -> 8 groups of n=512 tokens) -> out proj.

Sharding: by (batch, residue-group).  B*stride = 16 group-instances; each of
the 8 cores owns 2 (batch,group) pairs = 1024 tokens and computes their FULL
output rows (it holds all 16 heads for its tokens).  The residue groups are
independent, so there are no cross-device collectives at all; the host
permutes tokens into group-major order on the way in and inverts on the way
out.

Device kernel design (per core):
  - Host pre-transposes x so the kernel receives xT [E, 1024tok].
  - QKV: qT,kT produced feature-on-partition ([f,tok]); v produced
    token-on-partition ([tok,f]).  fp16 inputs, f32 PSUM.
  - scoresT[k,q] = kT.T-chunks @ qT per head; head pairs row-packed on the
    PE array (K=64 each at rows 0-63 / 64-127, separate PSUM banks).
  - exp on ScalarE without max-subtraction (scores are O(+-8), exp is safe).
    ACT runs ONLY Exp -> exactly one activation-table load.
  - PV: lhsT = [v | ones] (even heads) or [ones | v] (odd heads) so one
    accumulation chain yields o-rows plus 64 replicated softmax-denominator
    rows for free (matmul cost depends only on the moving free size).
  - softmax normalize: the denominator half of the PV PSUM is copied to the
    o-row partitions of an SBUF tile by a small partition-swap DMA (f32),
    one DVE reciprocal_approx_fast per pair, then two DVE multiplies.  No
    ACT ln/exp, no table thrash, no gpsimd broadcast.
  - out proj: lhsT = oT chunks, rhs = Wout rows -> natural [tok, E] output.
  - Software pipeline: scores(pair) and PV(pair) are separated in PE program
    order by an interleaved QKV/outproj unit so the in-order PE never waits
    on the ACT exp chain; DMAs are spread over the sync/gpsimd/vector queues.
"""

import os

import numpy as np

B, S, E = 2, 4096, 1024
H, ST = 16, 8
DH = E // H  # 64
N = S // ST  # 512 tokens per residue group
NCORES = 8
GPC = (B * ST) // NCORES  # 2 (batch,group) pairs per core
TOK = GPC * N  # 1024 tokens per core
P = 128
EC = E // P  # 8 contraction chunks of 128
NB = N // P  # 4 token chunks of 128 per group
FB = 2  # feature blocks of 512 in E
SCALE = 1.0 / float(np.sqrt(DH))

_CACHE: dict = {}


def _build_nc():
    import concourse.bass as bass
    import concourse.bacc as bacc
    import concourse.tile as tile
    from concourse import mybir

    F32 = mybir.dt.float32
    FP16 = mybir.dt.float16
    ADD = mybir.AluOpType.add
    EXP = mybir.ActivationFunctionType.Exp

    nc = bacc.Bacc()
    xt = nc.declare_dram_parameter("xt", [E, TOK], FP16, isOutput=False)
    wq = nc.declare_dram_parameter("wq", [EC, P, EC, P], FP16, isOutput=False)
    wk = nc.declare_dram_parameter("wk", [EC, P, EC, P], FP16, isOutput=False)
    wv = nc.declare_dram_parameter("wv", [E, E], FP16, isOutput=False)
    wo = nc.declare_dram_parameter("wo", [E, E], FP16, isOutput=False)
    bq = nc.declare_dram_parameter("bq", [E], F32, isOutput=False)
    bk = nc.declare_dram_parameter("bk", [E], F32, isOutput=False)
    bv = nc.declare_dram_parameter("bv", [E], F32, isOutput=False)
    bo = nc.declare_dram_parameter("bo", [E], F32, isOutput=False)
    out = nc.declare_dram_parameter("out", [TOK, E], F32, isOutput=True)

    with tile.TileContext(nc) as tc, (
        tc.tile_pool(name="const", bufs=1)
    ) as const, tc.tile_pool(name="xtp", bufs=5) as xtp, tc.tile_pool(
        name="wqkp", bufs=4
    ) as wqkp, tc.tile_pool(name="wvp", bufs=3) as wvp, tc.tile_pool(
        name="qtp", bufs=9
    ) as qtp, tc.tile_pool(name="ktp", bufs=9) as ktp, tc.tile_pool(
        name="vpp", bufs=5
    ) as vpp, tc.tile_pool(name="expp", bufs=4) as expp, tc.tile_pool(
        name="osbp", bufs=6
    ) as osbp, tc.tile_pool(name="recp", bufs=3) as recp, tc.tile_pool(
        name="rcfp", bufs=3
    ) as rcfp, tc.tile_pool(name="otp", bufs=17) as otp, tc.tile_pool(
        name="outp", bufs=3
    ) as outp, tc.tile_pool(name="psmm", bufs=2, space="PSUM") as psmm, tc.tile_pool(
        name="pssc", bufs=2, space="PSUM"
    ) as pssc, tc.tile_pool(name="pso", bufs=2, space="PSUM") as psop:
        # ---- constants (vector queue; off the sync/gpsimd critical path) --
        bq_sb = const.tile([P, EC], F32)
        nc.scalar.dma_start(out=bq_sb, in_=bq[:].rearrange("(c p) -> p c", p=P))
        bk_sb = const.tile([P, EC], F32)
        nc.scalar.dma_start(out=bk_sb, in_=bk[:].rearrange("(c p) -> p c", p=P))
        bv_bc = const.tile([P, E], F32)
        nc.scalar.dma_start(out=bv_bc, in_=bv[:].partition_broadcast(P))
        bo_bc = const.tile([P, E], F32)
        nc.scalar.dma_start(out=bo_bc, in_=bo[:].partition_broadcast(P))
        # Wout resident (fp16): [p, fb, dc, 512]; loaded mid-pipeline
        wo_sb = const.tile([P, FB, EC, 512], FP16)

        def load_wo():
            for fb in range(FB):
                nc.gpsimd.dma_start(
                    out=wo_sb[:, fb],
                    in_=wo[:, fb * 512 : (fb + 1) * 512].rearrange(
                        "(c p) f -> p c f", p=P
                    ),
                )

        xt_c = {0: [], 1: []}
        qts = {0: [], 1: []}
        kts = {0: [], 1: []}
        vts = {0: [], 1: []}
        ots = {0: [], 1: []}
        exs = {}
        recs = {}
        osbs = {0: {}, 1: {}}

        # xt: two batched DMAs per group ([P, 4, N], 4KB/partition) so the
        # QKV chain is not throttled by per-chunk DMA issue serialization
        def load_xt_g(g):
            eng = nc.sync if g == 0 else nc.gpsimd
            pieces = 2
            cpp = EC // pieces
            for piece in range(pieces):
                t = xtp.tile([P, cpp, N], FP16, tag="xt")
                eng.dma_start(
                    out=t,
                    in_=xt[
                        piece * cpp * P : (piece + 1) * cpp * P,
                        g * N : (g + 1) * N,
                    ].rearrange("(c p) n -> p c n", p=P),
                )
                for c in range(cpp):
                    xt_c[g].append(t[:, c, :])

        def emit_qk_ftile(g, which, ft):
            wmat, bias_sb, lst = (
                (wq, bq_sb, qts[g]) if which == "q" else (wk, bk_sb, kts[g])
            )
            wt = wqkp.tile([P, EC, P], FP16, tag="wqk")
            nc.gpsimd.dma_start(out=wt, in_=wmat[ft])
            ps = psmm.tile([P, N], F32, tag="mm")
            for c in range(EC):
                nc.tensor.matmul(
                    ps,
                    lhsT=wt[:, c, :],
                    rhs=xt_c[g][c],
                    start=(c == 0),
                    stop=(c == EC - 1),
                )
            if which == "q":
                t = qtp.tile([P, N], FP16, tag="qt")
            else:
                t = ktp.tile([P, N], FP16, tag="kt")
            nc.vector.tensor_scalar(
                out=t, in0=ps, scalar1=bias_sb[:, ft : ft + 1], scalar2=None, op0=ADD
            )
            lst.append(t)

        def emit_qk_unit(g, ft):
            emit_qk_ftile(g, "q", ft)
            emit_qk_ftile(g, "k", ft)

        def emit_v_fb(g, fb):
            if fb == 0:
                for tt in range(NB):
                    vt = vpp.tile([P, H, P], FP16, tag="vp")
                    # ones blocks (even heads: cols 64-127; odd: cols 0-63);
                    # the v-projection writes the v halves
                    v2 = vt.rearrange("p (h2 two) d -> p h2 two d", two=2)
                    nc.gpsimd.memset(v2[:, :, 0, DH:P], 1.0)
                    nc.gpsimd.memset(v2[:, :, 1, 0:DH], 1.0)
                    vts[g].append(vt)
            wvt = wvp.tile([P, EC, 512], FP16, tag="wv")
            nc.gpsimd.dma_start(
                out=wvt,
                in_=wv[:, fb * 512 : (fb + 1) * 512].rearrange(
                    "(c p) f -> p c f", p=P
                ),
            )
            wv_ts = [wvt[:, c, :] for c in range(EC)]
            for tt in range(NB):
                ps = psmm.tile([P, 512], F32, tag="mm")
                for c in range(EC):
                    nc.tensor.matmul(
                        ps,
                        lhsT=xt_c[g][c][:, tt * P : (tt + 1) * P],
                        rhs=wv_ts[c],
                        start=(c == 0),
                        stop=(c == EC - 1),
                    )
                # batched bias-add + scatter into the per-head v layout:
                # heads fb*8+hl; even hl -> v at cols 0:64, odd -> 64:128.
                vt = vts[g][tt]
                v2 = vt.rearrange("p (h2 two) d -> p h2 two d", two=2)
                p2 = ps.rearrange("p (f2 two d) -> p f2 two d", two=2, d=DH)
                b2 = bv_bc[:, fb * 512 : (fb + 1) * 512].rearrange(
                    "p (f2 two d) -> p f2 two d", two=2, d=DH
                )
                h2lo = fb * (H // 4)
                h2hi = (fb + 1) * (H // 4)
                nc.vector.tensor_add(
                    out=v2[:, h2lo:h2hi, 0, 0:DH],
                    in0=p2[:, :, 0, :],
                    in1=b2[:, :, 0, :],
                )
                nc.vector.tensor_add(
                    out=v2[:, h2lo:h2hi, 1, DH:P],
                    in0=p2[:, :, 1, :],
                    in1=b2[:, :, 1, :],
                )

        def emit_scores_pair(g, pr):
            # scores + exp for both heads of the pair, row-packed on the PE
            # array (K=64 each at rows 0-63 / 64-127, separate PSUM banks)
            ex_AB = {}
            for h in (2 * pr, 2 * pr + 1):
                ex_AB[h] = expp.tile([P, NB, N], FP16, tag="exp", name=f"ex{g}_{h}")
            for half in range(2):
                scs = {}
                for h in (2 * pr, 2 * pr + 1):
                    lo, hi = (0, DH) if h % 2 == 0 else (DH, P)
                    sc = pssc.tile([P, 2, N], F32, tag="sc")
                    for cc in range(2):
                        c = 2 * half + cc
                        nc.tensor.matmul(
                            sc[:, cc],
                            lhsT=kts[g][pr][lo:hi, c * P : (c + 1) * P],
                            rhs=qts[g][pr][lo:hi, :],
                            start=True,
                            stop=True,
                        )
                    scs[h] = sc
                for h in (2 * pr, 2 * pr + 1):
                    nc.scalar.activation(
                        out=ex_AB[h][:, 2 * half : 2 * half + 2],
                        in_=scs[h],
                        func=EXP,
                    )
            exs[(g, pr)] = ex_AB

        def emit_pv_pair(g, pr):
            ex_AB = exs.pop((g, pr))
            for h in (2 * pr, 2 * pr + 1):
                ex = ex_AB[h]
                po = psop.tile([P, N], F32, tag="po")
                for c in range(NB):
                    nc.tensor.matmul(
                        po,
                        lhsT=vts[g][c][:, h, :],
                        rhs=ex[:, c, :],
                        start=(c == 0),
                        stop=(c == NB - 1),
                    )
                osb = osbp.tile([P, N], F32, tag="osb")
                nc.vector.tensor_copy(out=osb, in_=po)
                osbs[g][h] = osb

        def emit_swap_pair(g, pr):
            # partition-swap DMAs: move the replicated denominator rows onto
            # the o-row partitions of the rec tile (SBUF -> SBUF).  Emitted a
            # full pair-slot after the evacuations so the in-order sync queue
            # head never blocks waiting for them.
            rec = recp.tile([P, N], F32, tag="rec")
            nc.sync.dma_start(out=rec[0:DH, :], in_=osbs[g][2 * pr][DH:P, :])
            nc.sync.dma_start(out=rec[DH:P, :], in_=osbs[g][2 * pr + 1][0:DH, :])
            recs[(g, pr)] = rec

        def emit_norm_pair(g, pr):
            # reciprocal on DVE; the normalize multiplies run on the mostly
            # idle GpSimd engine so they never serialize the DVE stream
            rec = recs.pop((g, pr))
            rcf = rcfp.tile([P, N], F32, tag="rcf")
            nc.vector.reciprocal_approx_fast(out=rcf, in_=rec)
            ot = otp.tile([P, N], FP16, tag="ot")
            he, ho = 2 * pr, 2 * pr + 1
            nc.gpsimd.tensor_mul(
                out=ot[0:DH, :], in0=osbs[g][he][0:DH, :], in1=rcf[0:DH, :]
            )
            nc.gpsimd.tensor_mul(
                out=ot[DH:P, :], in0=osbs[g][ho][DH:P, :], in1=rcf[DH:P, :]
            )
            del osbs[g][he], osbs[g][ho]
            ots[g].append(ot)

        def emit_slot(g, pr, mids=()):
            # one software-pipeline slot: scores(pr), then the previous
            # pair's swap DMAs (their evac deps resolved last slot, so the
            # sync queue head never blocks), the interleaved PE units, the
            # previous pair's normalize (its swap completed mid-slot), and
            # finally PV(pr), which waits on this pair's exp chain that ran
            # under the interleaved units.
            emit_scores_pair(g, pr)
            if pr > 0:
                emit_swap_pair(g, pr - 1)
            for m in mids:
                m()
            if pr > 0:
                emit_norm_pair(g, pr - 1)
            emit_pv_pair(g, pr)

        def emit_outproj_unit(g, fb, tt):
            ps = psmm.tile([P, 512], F32, tag="mm")
            for dc in range(EC):
                nc.tensor.matmul(
                    ps,
                    lhsT=ots[g][dc][:, tt * P : (tt + 1) * P],
                    rhs=wo_sb[:, fb, dc, :],
                    start=(dc == 0),
                    stop=(dc == EC - 1),
                )
            ob = outp.tile([P, 512], F32, tag="ob")
            nc.vector.tensor_add(
                out=ob, in0=ps, in1=bo_bc[:, fb * 512 : (fb + 1) * 512]
            )
            # stores on the sync queue: it is idle at the tail, so the final
            # per-engine drain does not wait behind the gpsimd DMA backlog
            nc.sync.dma_start(
                out=out[
                    g * N + tt * P : g * N + (tt + 1) * P,
                    fb * 512 : (fb + 1) * 512,
                ],
                in_=ob,
            )

        # ---- software-pipelined program order --------------------------
        load_xt_g(0)
        emit_qk_unit(0, 0)
        emit_qk_unit(0, 1)
        emit_slot(0, 0, [lambda: emit_v_fb(0, 0)])
        emit_slot(0, 1, [lambda: emit_qk_unit(0, 2)])
        emit_slot(0, 2, [lambda: emit_qk_unit(0, 3)])
        emit_slot(0, 3, [lambda: emit_qk_unit(0, 4), lambda: emit_qk_unit(0, 5)])
        emit_slot(0, 4, [lambda: load_xt_g(1), lambda: emit_v_fb(0, 1)])
        emit_slot(0, 5, [lambda: emit_qk_unit(0, 6), load_wo])
        emit_slot(0, 6, [lambda: emit_qk_unit(0, 7)])
        emit_slot(0, 7, [lambda: emit_qk_unit(1, 0)])
        # group-0 tail: swap+norm un-deferred (nothing else to overlap)
        emit_qk_unit(1, 1)
        emit_swap_pair(0, 7)
        emit_qk_unit(1, 2)
        emit_norm_pair(0, 7)
        # group-1 qk projection (PE-dense, ACT idle)
        for ft in range(3, EC):
            emit_qk_unit(1, ft)
        # group-1 attention interleaved with group-0 out proj
        emit_slot(1, 0, [lambda: emit_v_fb(1, 0)])
        emit_slot(1, 1, [lambda: emit_outproj_unit(0, 0, 0)])
        emit_slot(1, 2, [lambda: emit_outproj_unit(0, 0, 1)])
        emit_slot(
            1,
            3,
            [lambda: emit_outproj_unit(0, 0, 2), lambda: emit_outproj_unit(0, 0, 3)],
        )
        emit_slot(1, 4, [lambda: emit_v_fb(1, 1)])
        emit_slot(
            1,
            5,
            [lambda: emit_outproj_unit(0, 1, 0), lambda: emit_outproj_unit(0, 1, 1)],
        )
        emit_slot(1, 6, [lambda: emit_outproj_unit(0, 1, 2)])
        emit_slot(1, 7, [lambda: emit_outproj_unit(0, 1, 3)])
        # group-1 tail: swap+norm un-deferred
        emit_swap_pair(1, 7)
        emit_norm_pair(1, 7)
        for fb in range(FB):
            for tt in range(NB):
                emit_outproj_unit(1, fb, tt)
    nc.finalize()
    return nc


def _get_nc():
    if "nc" not in _CACHE:
        _CACHE["nc"] = _build_nc()
    return _CACHE["nc"]


def _make_in_maps(x, Wqkv, bqkv, Wout, bout):
    """Host-side sharding: permute tokens to group-major, pre-transpose x."""
    x = np.asarray(x, dtype=np.float32)
    Wqkv = np.asarray(Wqkv, dtype=np.float32)
    bqkv = np.asarray(bqkv, dtype=np.float32)
    Wout = np.ascontiguousarray(np.asarray(Wout, dtype=np.float16))
    bout = np.ascontiguousarray(np.asarray(bout, dtype=np.float32))

    # group-major token order: x_perm[b, g*N + i] = x[b, i*ST + g]
    x_perm = x.reshape(B, N, ST, E).transpose(0, 2, 1, 3)  # [B, ST, N, E]

    # [E, E] -> [ft, p, c, f] tile-major so each SBUF partition reads big runs
    def tile_qk(w):
        return np.ascontiguousarray(
            w.reshape(EC, P, EC, P).transpose(2, 1, 0, 3).astype(np.float16)
        )

    wq = tile_qk(Wqkv[:, 0:E] * SCALE)
    wk = tile_qk(Wqkv[:, E : 2 * E])
    wv = np.ascontiguousarray(Wqkv[:, 2 * E : 3 * E].astype(np.float16))
    bq = np.ascontiguousarray(bqkv[0:E] * SCALE)
    bk = np.ascontiguousarray(bqkv[E : 2 * E])
    bv = np.ascontiguousarray(bqkv[2 * E : 3 * E])

    in_maps = []
    for c in range(NCORES):
        b = c // (NCORES // B)
        g0 = GPC * (c % (NCORES // B))
        xc = x_perm[b, g0 : g0 + GPC].reshape(TOK, E)  # [1024, E]
        xct = np.ascontiguousarray(xc.T.astype(np.float16))  # [E, 1024]
        in_maps.append(
            {
                "xt": xct,
                "wq": wq,
                "wk": wk,
                "wv": wv,
                "wo": Wout,
                "bq": bq,
                "bk": bk,
                "bv": bv,
                "bo": bout,
            }
        )
    return in_maps


def kernel(x, Wqkv, bqkv, Wout, bout):
    from concourse.bass_utils import run_bass_kernel_spmd

    nc = _get_nc()
    in_maps = _make_in_maps(x, Wqkv, bqkv, Wout, bout)
    trace = bool(int(os.environ.get("KERNEL_TRACE", "0")))
    res = run_bass_kernel_spmd(
        nc, in_maps, core_ids=list(range(NCORES)), trace=trace
    )
    _CACHE["last_result"] = res

    # reassemble: core outputs are [1024 tok, E] in group-major token order
    out = np.empty((B, S, E), dtype=np.float32)
    for b in range(B):
        per_b = [res.results[b * (NCORES // B) + j]["out"] for j in range(NCORES // B)]
        perm = np.concatenate(per_b, axis=0)  # [ST*N, E] group-major
        out[b] = perm.reshape(ST, N, E).transpose(1, 0, 2).reshape(S, E)
    return out
